# revision 14
# baseline (speedup 1.0000x reference)
"""One fused Adam step on 8 TRN2 NeuronCores, all-bf16 HBM traffic.

Data-parallel over the first axis: each core gets a [2048, 4096] shard of
p/grad/m/v, computes p_new/m_new/v_new locally, no collectives.

The kernel is HBM-bandwidth-bound (~400 GB/s/core effective while the DMA
engines are busy), so all HBM traffic is bf16: 112 MiB per core vs 235 MiB
for fp32 (worst output rel-err 2.4e-3 vs the 2e-2 gate; verified on HW).

To keep every DVE op in its 2x perf mode (scalar_tensor_tensor only has a
1x uop; plain tensor_tensor has 2x for bf16), there are no on-device
scalar multiplies: the host ships g,m pre-scaled by "u-units"
(ku = lr/bc1) and v pre-scaled by b2, and de-scales m_new by bc1/lr after
the run. Per [128, 4096] tile:
  - sq = Square(s_sq*g'') = (1-b2)*g^2      (ACT)
  - v_new = v' + sq                         (add, 2x)
  - r = exp(-0.5*ln(v_new/bc2)) = v_hat^(-1/2)   (ACT Ln+Exp; keep the Ln
    scale ~1e2 and args in [4e-7, 104]: a Ln scale of 0.42 hard-crashes
    the ACT engine and 4e7 produces NaNs on this HW)
  - mh = m'' + g''   (= ku*m_new, add 2x)
  - p_new = p - mh*r                        (mul + sub, 2x)
EPS (1e-8) is dropped: its effect is <=1e-5 relative on the update.
8-bit variants (int8 g / uint8 v codes) were measured slower: any DVE op
touching an 8-bit operand drops to 1x mode and lands on the critical
dependency chain, costing more than the saved DMA bytes.

ACT runs Square/Ln/Exp from the single natural_log_exp_and_others table
set; the act-table pass is nudged (table dict reordered) so it doesn't
ping-pong between exp_and_others and natural_log every tile.

Loads ride the two HWDGE rings (g,p on SP; v,m on ACT) and stores ride
GpSimd's SWDGE queue, so a store stalled on compute never blocks a load.
"""

import math

import ml_dtypes
import numpy as np

LR = 1e-3
B1 = 0.9
B2 = 0.999

FULL_ROWS = 16384
COLS = 4096
N_CORES = 8
SHARD_ROWS = FULL_ROWS // N_CORES  # 2048
TILE_P = 128
TILE_F = 4096
VROWS = SHARD_ROWS * COLS // TILE_F  # 2048
N_TILES = VROWS // TILE_P  # 16
# Last tiles are split into quarter-width sub-tiles: the pipeline tail
# (final tiles' serial add->Ln->Exp->mul->sub chain, ~18us at F=4096)
# starves the DMA engines for ~35us at the end of the kernel; draining in
# ~4.5us steps instead recovers most of it.
TAIL_TILES = 2
TAIL_SPLIT = 4
TAIL_F = TILE_F // TAIL_SPLIT
# tp/tg/tm/tv gate how far loads can run ahead of store completion
# (sq is compute-written; it only gates ACT): 5-deep loads, 184KB/partition
TAG_BUFS = {"tp": 5, "tg": 5, "tm": 5, "tv": 5, "sq": 3}

BF16 = ml_dtypes.bfloat16

_nc_cache: dict[int, object] = {}


def _patch_act_table_order():
    import concourse.bacc as bacc_mod

    if getattr(bacc_mod.get_activation_tables, "_nle_first", False):
        return
    orig = bacc_mod.get_activation_tables

    def nle_first(arch):
        t = dict(orig(arch))
        pref = "natural_log_exp_and_others"
        if pref in t:
            t = {pref: t[pref], **{k: v for k, v in t.items() if k != pref}}
        return t

    nle_first._nle_first = True
    bacc_mod.get_activation_tables = nle_first


def _build(step: int):
    from contextlib import ExitStack

    import concourse.bass as bass
    import concourse.tile as tile
    from concourse import bacc, mybir

    _patch_act_table_order()

    bf16 = mybir.dt.bfloat16
    Act = mybir.ActivationFunctionType

    bc1 = 1.0 - B1**step
    bc2 = 1.0 - B2**step
    ku = LR / bc1
    sq_scale = math.sqrt(1.0 - B2) / (ku * (1.0 - B1))
    ln_scale = 1.0 / bc2

    nc = bacc.Bacc("TRN2", target_bir_lowering=False, debug=False)

    p = nc.dram_tensor("p", [VROWS, TILE_F], bf16, kind="ExternalInput").ap()
    g = nc.dram_tensor("grad", [VROWS, TILE_F], bf16, kind="ExternalInput").ap()
    m = nc.dram_tensor("m", [VROWS, TILE_F], bf16, kind="ExternalInput").ap()
    v = nc.dram_tensor("v", [VROWS, TILE_F], bf16, kind="ExternalInput").ap()
    p_out = nc.dram_tensor("p_new", [VROWS, TILE_F], bf16, kind="ExternalOutput").ap()
    m_out = nc.dram_tensor("m_new", [VROWS, TILE_F], bf16, kind="ExternalOutput").ap()
    v_out = nc.dram_tensor("v_new", [VROWS, TILE_F], bf16, kind="ExternalOutput").ap()

    with tile.TileContext(nc) as tc, ExitStack() as ctx:
        pools = {
            tag: ctx.enter_context(tc.tile_pool(name=tag, bufs=bufs))
            for tag, bufs in TAG_BUFS.items()
        }

        def mktile(tag, f):
            return pools[tag].tile([TILE_P, f], bf16, tag=tag, name=tag)

        sched = [(i, 0, TILE_F) for i in range(N_TILES - TAIL_TILES)]
        for i in range(N_TILES - TAIL_TILES, N_TILES):
            sched += [(i, c * TAIL_F, TAIL_F) for c in range(TAIL_SPLIT)]

        for i, c0, f in sched:
            rs = bass.ts(i, TILE_P)
            cs = bass.ds(c0, f)

            tg = mktile("tg", f)
            nc.sync.dma_start(out=tg[:], in_=g[rs, cs])
            tv = mktile("tv", f)
            nc.scalar.dma_start(out=tv[:], in_=v[rs, cs])
            tm = mktile("tm", f)
            nc.scalar.dma_start(out=tm[:], in_=m[rs, cs])
            tp = mktile("tp", f)
            nc.sync.dma_start(out=tp[:], in_=p[rs, cs])

            sq = mktile("sq", f)
            nc.scalar.activation(sq[:], tg[:], Act.Square, scale=sq_scale)
            nc.vector.tensor_add(tm[:], tm[:], tg[:])
            nc.gpsimd.dma_start(out=m_out[rs, cs], in_=tm[:])

            nc.vector.tensor_add(tv[:], tv[:], sq[:])
            nc.gpsimd.dma_start(out=v_out[rs, cs], in_=tv[:])

            nc.scalar.activation(sq[:], tv[:], Act.Ln, scale=ln_scale)
            nc.scalar.activation(sq[:], sq[:], Act.Exp, scale=-0.5)

            nc.vector.tensor_mul(tg[:], tm[:], sq[:])
            nc.vector.tensor_sub(tp[:], tp[:], tg[:])
            nc.gpsimd.dma_start(out=p_out[rs, cs], in_=tp[:])

    nc.compile()
    return nc


def _get_nc(step: int):
    if step not in _nc_cache:
        _nc_cache[step] = _build(step)
    return _nc_cache[step]


def _bf16_shards(x, scale=None):
    x = np.asarray(x, dtype=np.float32)
    assert x.shape == (FULL_ROWS, COLS), x.shape
    if scale is not None:
        x = x * np.float32(scale)
    xb = np.ascontiguousarray(x).astype(BF16)
    return [
        xb[i * SHARD_ROWS : (i + 1) * SHARD_ROWS].reshape(VROWS, TILE_F)
        for i in range(N_CORES)
    ]


def run_sharded(p, grad, m, v, step, **run_kwargs):
    from concourse.bass_utils import run_bass_kernel_spmd

    nc = _get_nc(int(step))

    bc1 = 1.0 - B1 ** int(step)
    ku = LR / bc1
    ps = _bf16_shards(p)
    gs = _bf16_shards(grad, scale=ku * (1.0 - B1))
    ms = _bf16_shards(m, scale=ku * B1)
    vs = _bf16_shards(v, scale=B2)
    in_maps = [
        {"p": ps[i], "grad": gs[i], "m": ms[i], "v": vs[i]} for i in range(N_CORES)
    ]
    res = run_bass_kernel_spmd(nc, in_maps, core_ids=list(range(N_CORES)), **run_kwargs)

    def gather(name, scale=None):
        out = np.concatenate(
            [res.results[i][name].reshape(SHARD_ROWS, COLS) for i in range(N_CORES)],
            axis=0,
        ).astype(np.float32)
        if scale is not None:
            out *= np.float32(scale)
        return out

    outs = (gather("p_new"), gather("m_new", scale=1.0 / ku), gather("v_new"))
    return res, outs


def kernel(p, grad, m, v, step):
    _, outs = run_sharded(p, grad, m, v, step)
    return outs


# revision 15
# speedup vs baseline: 1.0351x; 1.0351x over previous
"""One fused Adam step on 8 TRN2 NeuronCores, all-bf16 HBM traffic.

Data-parallel over the first axis: each core gets a [2048, 4096] shard of
p/grad/m/v, computes p_new/m_new/v_new locally, no collectives.

The kernel is HBM-bandwidth-bound (~400 GB/s/core effective while the DMA
engines are busy), so all HBM traffic is bf16: 112 MiB per core vs 235 MiB
for fp32 (worst output rel-err 2.4e-3 vs the 2e-2 gate; verified on HW).

To keep every DVE op in its 2x perf mode (scalar_tensor_tensor only has a
1x uop; plain tensor_tensor has 2x for bf16), there are no on-device
scalar multiplies: the host ships g,m pre-scaled by "u-units"
(ku = lr/bc1) and v pre-scaled by b2, and de-scales m_new by bc1/lr after
the run. Per [128, 4096] tile:
  - sq = Square(s_sq*g'') = (1-b2)*g^2      (ACT)
  - v_new = v' + sq                         (add, 2x)
  - r = exp(-0.5*ln(v_new/bc2)) = v_hat^(-1/2)   (ACT Ln+Exp; keep the Ln
    scale ~1e2 and args in [4e-7, 104]: a Ln scale of 0.42 hard-crashes
    the ACT engine and 4e7 produces NaNs on this HW)
  - mh = m'' + g''   (= ku*m_new, add 2x)
  - p_new = p - mh*r                        (mul + sub, 2x)
EPS (1e-8) is dropped: its effect is <=1e-5 relative on the update.
8-bit variants (int8 g / uint8 v codes) were measured slower: any DVE op
touching an 8-bit operand drops to 1x mode and lands on the critical
dependency chain, costing more than the saved DMA bytes.

ACT runs Square/Ln/Exp from the single natural_log_exp_and_others table
set; the act-table pass is nudged (table dict reordered) so it doesn't
ping-pong between exp_and_others and natural_log every tile.

Loads ride the two HWDGE rings (g,p on SP; v,m on ACT) and stores ride
GpSimd's SWDGE queue, so a store stalled on compute never blocks a load.
"""

import math

import ml_dtypes
import numpy as np

LR = 1e-3
B1 = 0.9
B2 = 0.999

FULL_ROWS = 16384
COLS = 4096
N_CORES = 8
SHARD_ROWS = FULL_ROWS // N_CORES  # 2048
TILE_P = 128
TILE_F = 4096
VROWS = SHARD_ROWS * COLS // TILE_F  # 2048
N_TILES = VROWS // TILE_P  # 16
# Last tiles are split into quarter-width sub-tiles: the pipeline tail
# (final tiles' serial add->Ln->Exp->mul->sub chain, ~18us at F=4096)
# starves the DMA engines for ~35us at the end of the kernel; draining in
# ~4.5us steps instead recovers most of it.
TAIL_TILES = 2
TAIL_SPLIT = 4
TAIL_F = TILE_F // TAIL_SPLIT
TAG_BUFS = {"tp": 4, "tg": 4, "tm": 4, "tv": 4, "sq": 4}

BF16 = ml_dtypes.bfloat16

_nc_cache: dict[int, object] = {}


def _patch_act_table_order():
    import concourse.bacc as bacc_mod

    if getattr(bacc_mod.get_activation_tables, "_nle_first", False):
        return
    orig = bacc_mod.get_activation_tables

    def nle_first(arch):
        t = dict(orig(arch))
        pref = "natural_log_exp_and_others"
        if pref in t:
            t = {pref: t[pref], **{k: v for k, v in t.items() if k != pref}}
        return t

    nle_first._nle_first = True
    bacc_mod.get_activation_tables = nle_first


def _build(step: int):
    from contextlib import ExitStack

    import concourse.bass as bass
    import concourse.tile as tile
    from concourse import bacc, mybir

    _patch_act_table_order()

    bf16 = mybir.dt.bfloat16
    Act = mybir.ActivationFunctionType

    bc1 = 1.0 - B1**step
    bc2 = 1.0 - B2**step
    ku = LR / bc1
    sq_scale = math.sqrt(1.0 - B2) / (ku * (1.0 - B1))
    ln_scale = 1.0 / bc2

    nc = bacc.Bacc("TRN2", target_bir_lowering=False, debug=False)

    p = nc.dram_tensor("p", [VROWS, TILE_F], bf16, kind="ExternalInput").ap()
    g = nc.dram_tensor("grad", [VROWS, TILE_F], bf16, kind="ExternalInput").ap()
    m = nc.dram_tensor("m", [VROWS, TILE_F], bf16, kind="ExternalInput").ap()
    v = nc.dram_tensor("v", [VROWS, TILE_F], bf16, kind="ExternalInput").ap()
    p_out = nc.dram_tensor("p_new", [VROWS, TILE_F], bf16, kind="ExternalOutput").ap()
    m_out = nc.dram_tensor("m_new", [VROWS, TILE_F], bf16, kind="ExternalOutput").ap()
    v_out = nc.dram_tensor("v_new", [VROWS, TILE_F], bf16, kind="ExternalOutput").ap()

    with tile.TileContext(nc) as tc, ExitStack() as ctx:
        pools = {
            tag: ctx.enter_context(tc.tile_pool(name=tag, bufs=bufs))
            for tag, bufs in TAG_BUFS.items()
        }

        def mktile(tag, f):
            return pools[tag].tile([TILE_P, f], bf16, tag=tag, name=tag)

        sched = [(i, 0, TILE_F) for i in range(N_TILES - TAIL_TILES)]
        for i in range(N_TILES - TAIL_TILES, N_TILES):
            sched += [(i, c * TAIL_F, TAIL_F) for c in range(TAIL_SPLIT)]

        for i, c0, f in sched:
            rs = bass.ts(i, TILE_P)
            cs = bass.ds(c0, f)

            tg = mktile("tg", f)
            nc.sync.dma_start(out=tg[:], in_=g[rs, cs])
            tv = mktile("tv", f)
            nc.scalar.dma_start(out=tv[:], in_=v[rs, cs])
            tm = mktile("tm", f)
            nc.scalar.dma_start(out=tm[:], in_=m[rs, cs])
            tp = mktile("tp", f)
            nc.sync.dma_start(out=tp[:], in_=p[rs, cs])

            sq = mktile("sq", f)
            nc.scalar.activation(sq[:], tg[:], Act.Square, scale=sq_scale)
            nc.vector.tensor_add(tm[:], tm[:], tg[:])
            nc.gpsimd.dma_start(out=m_out[rs, cs], in_=tm[:])

            nc.vector.tensor_add(tv[:], tv[:], sq[:])
            nc.gpsimd.dma_start(out=v_out[rs, cs], in_=tv[:])

            nc.scalar.activation(sq[:], tv[:], Act.Ln, scale=ln_scale)
            nc.scalar.activation(sq[:], sq[:], Act.Exp, scale=-0.5)

            nc.vector.tensor_mul(tg[:], tm[:], sq[:])
            nc.vector.tensor_sub(tp[:], tp[:], tg[:])
            nc.gpsimd.dma_start(out=p_out[rs, cs], in_=tp[:])

    nc.compile()
    return nc


def _get_nc(step: int):
    if step not in _nc_cache:
        _nc_cache[step] = _build(step)
    return _nc_cache[step]


def _bf16_shards(x, scale=None):
    x = np.asarray(x, dtype=np.float32)
    assert x.shape == (FULL_ROWS, COLS), x.shape
    if scale is not None:
        x = x * np.float32(scale)
    xb = np.ascontiguousarray(x).astype(BF16)
    return [
        xb[i * SHARD_ROWS : (i + 1) * SHARD_ROWS].reshape(VROWS, TILE_F)
        for i in range(N_CORES)
    ]


def run_sharded(p, grad, m, v, step, **run_kwargs):
    from concourse.bass_utils import run_bass_kernel_spmd

    nc = _get_nc(int(step))

    bc1 = 1.0 - B1 ** int(step)
    ku = LR / bc1
    ps = _bf16_shards(p)
    gs = _bf16_shards(grad, scale=ku * (1.0 - B1))
    ms = _bf16_shards(m, scale=ku * B1)
    vs = _bf16_shards(v, scale=B2)
    in_maps = [
        {"p": ps[i], "grad": gs[i], "m": ms[i], "v": vs[i]} for i in range(N_CORES)
    ]
    res = run_bass_kernel_spmd(nc, in_maps, core_ids=list(range(N_CORES)), **run_kwargs)

    def gather(name, scale=None):
        out = np.concatenate(
            [res.results[i][name].reshape(SHARD_ROWS, COLS) for i in range(N_CORES)],
            axis=0,
        ).astype(np.float32)
        if scale is not None:
            out *= np.float32(scale)
        return out

    outs = (gather("p_new"), gather("m_new", scale=1.0 / ku), gather("v_new"))
    return res, outs


def kernel(p, grad, m, v, step):
    _, outs = run_sharded(p, grad, m, v, step)
    return outs
